# revision 17
# baseline (speedup 1.0000x reference)
# ISTA dictionary-learning forward pass on 8 Trainium2 NeuronCores.
#
# Math (matching the reference):
#   p   = unfold(y, 8x8 patches, stride 4), per-patch mean removed
#   A   = l2-normalized atoms [256, 192];  X = A A^T;  L = ||X||_2
#   q   = A p^T;  25x ISTA:  c <- S_thr((I - X/L) c + q/L),  thr = 0.1/L
#   rec = c^T A + mean;  out = fold(rec) / counts
#
# Distribution: data-parallel over the batch axis - core b processes image b.
#
# Device-side formulation (per core, one image, Hp=Wp=63 patch grid padded
# to a 64x64 grid, N=4096 columns):
#   - no explicit unfold: y is space-to-depth'd on host into fp16
#     y4d[97, F]: rows 0..47 = Y4[(c,r,s),(u,v)] (h=4u+r, w=4v+s), row 48 =
#     per-patch mean, rows 49..96 = rows 0..47 shifted one column. q = A p^T
#     then needs only TWO K=97 fp16 matmuls per 512 cols (quadrant pairs
#     packed along K; the rank-1 mean correction rides as row 48 of the
#     first lhsT).
#   - iteration 1 is matmul-free: c1 = S_thr(q/L) straight from the q-phase
#     PSUM. qs copies PSUM->SBUF and the per-iteration qs->PSUM seeds of the
#     ACT-path chunks ride on DMA queues, not engines.
#   - ISTA iterations: K=256 contraction split in 2; stationary M2 and
#     moving c in fp16; soft-threshold is ONE fused custom DVE instruction
#     per chunk (6/8), the rest via ACT relu-pair + GPSIMD subtract reading
#     qs-seeded PSUM.
#   - rec^T = A^T c via fp16 matmuls whose output d-axis is pre-permuted;
#     rec PSUM chunks DMA straight to DRAM. The fold (overlap-add), the
#     per-patch mean add and the counts divide all happen on host.

import numpy as np

ATOM, STRIDE, NBA, LMBDA = 8, 4, 256, 0.1
NITER = 25
B, C, H, W = 8, 3, 256, 256
D = 192
G = 64                 # padded patch grid (ph, pw in [0, 64))
NCOL = G * G           # 4096 padded patch columns per core
Y4F = 66 * 64          # y4 free size: max AP offset is 3584+64+512 = 4160
NCORES = 8

_prog_cache = {}


# ---------------------------------------------------------------- custom op
def _softshrink_op():
    """out = t - clamp(t, s0, s1) with t = in0 + in1, as one DVE instruction."""
    import concourse.dve_ops as dve_ops
    from concourse.dve_spec import Spec, Src0, Src1, C0, C1, lower, maxx, minn
    from concourse.dve_uop import DveOpSpec

    name = "SOFTSHRINK_ADD_ANT"
    for op in dve_ops.OPS:
        if op.name == name:
            return op

    def ref(in0, in1, s0, s1, imm2):
        t = in0.astype(np.float32) + in1
        return t - np.clip(t, s0, s1)

    t = Src0 + Src1
    spec = Spec(body=t - minn(maxx(t, C0), C1), reference=ref)
    row = dve_ops._CUSTOM_DVE_ROW_BASE + len(dve_ops.OPS)
    shas = {}
    for ver in ("v3", "v4"):
        uops = lower(spec, ver=ver)
        shas[ver] = DveOpSpec(name=name, opcode=row, uops=uops, rd1_en=True).sha(ver)
    op = dve_ops.DveOp(name, spec, subdim=False, uops_sha=shas)
    dve_ops.OPS.append(op)
    dve_ops.CUSTOM_DVE_SPECS[name] = spec
    dve_ops._SUB_OPCODE_FOR_NAME[name] = row
    return op


# ---------------------------------------------------------------- host packing
def _host_constants(atoms):
    A = atoms.reshape(NBA, D).astype(np.float64)
    An = A / np.linalg.norm(A, axis=1, keepdims=True)
    X = An @ An.T
    L = float(np.linalg.norm(X, 2))
    thr = LMBDA / L
    M2 = np.eye(NBA) - X / L
    arow = An.sum(1)
    An32 = An.astype(np.float32)

    # M2 lhsT tiles: m2p[p, (kc*2+mb)*128+m] = M2[kc*128+p, mb*128+m]
    m2p = np.zeros((128, 512), np.float16)
    for kc in range(2):
        for mb in range(2):
            m2p[:, (kc * 2 + mb) * 128:(kc * 2 + mb + 1) * 128] = \
                M2[kc * 128:(kc + 1) * 128, mb * 128:(mb + 1) * 128].astype(np.float16)

    # q-phase lhsT [97, 2*256], quadrant PAIRS packed along K:
    #   block A (quads 0,1):  rows 0..47 = An[k,c,0+r,0+s]/L, row 48 = -arow/L,
    #                         rows 49..96 = An[k,c,0+r,4+s]/L
    #   block B (quads 2,3):  rows 0..47 = An[k,c,4+r,0+s]/L, row 48 = 0,
    #                         rows 49..96 = An[k,c,4+r,4+s]/L
    #   qlhs2[:, (blk*2+mb)*128+m] covers atoms mb*128+m.
    # rows 97..127 stay zero: K padded to 128 so LDW-opt accepts the load
    An4 = (An32 / L).reshape(NBA, 3, 8, 8)
    qlhs2 = np.zeros((128, 512), np.float16)
    for blk, a in enumerate(range(2)):
        for b in range(2):
            blk48 = An4[:, :, 4 * a:4 * a + 4, 4 * b:4 * b + 4] \
                .transpose(1, 2, 3, 0).reshape(48, 256)
            r0 = 0 if b == 0 else 49
            for mb in range(2):
                qlhs2[r0:r0 + 48, (a * 2 + mb) * 128:(a * 2 + mb + 1) * 128] = \
                    blk48[:, mb * 128:(mb + 1) * 128].astype(np.float16)
    qlhs2[48, 0:128] = (-(arow[0:128] / L)).astype(np.float16)
    qlhs2[48, 128:256] = (-(arow[128:256] / L)).astype(np.float16)

    # rec lhsT with permuted d-axis: d'(a=mb, m): b=m//48; t=m%48;
    #   r=t//12; s=(t//3)%4; c=t%3; d = c*64 + (4a+r)*8 + (4b+s)
    dprime = np.zeros((2, 96), np.int64)
    for a in range(2):
        for m in range(96):
            b = m // 48
            t = m % 48
            r, s, c = t // 12, (t // 3) % 4, t % 3
            dprime[a, m] = c * 64 + (4 * a + r) * 8 + (4 * b + s)
    anrec = np.zeros((128, 4 * 96), np.float16)
    for kc in range(2):
        for mb in range(2):
            anrec[:, (kc * 2 + mb) * 96:(kc * 2 + mb + 1) * 96] = \
                An32[kc * 128:(kc + 1) * 128][:, dprime[mb]].astype(np.float16)

    # separable overlap counts
    cnt1 = np.zeros(H, np.float64)
    for ph in range(63):
        cnt1[4 * ph:4 * ph + 8] += 1
    counts = np.outer(cnt1, cnt1).astype(np.float32)

    return dict(m2p=m2p, qlhs2=qlhs2, anrec=anrec, counts=counts, thr=thr)


def _make_y4(img):
    """[3,256,256] -> (y4d [97, Y4F] fp16, mg [64,64] f32).

    Rows 0..47: Y4[(c*16+r*4+s), u*64+v] = img[c, 4u+r, 4v+s].
    Row 48: per-patch mean on the padded (ph, pw) grid (0 outside).
    Rows 49..96: rows 0..47 shifted one column left (for K-packed quads)."""
    t = img.reshape(3, 64, 4, 64, 4).transpose(0, 2, 4, 1, 3).reshape(48, 4096)
    out = np.zeros((97, Y4F), np.float16)
    out[:48, :4096] = t.astype(np.float16)
    out[49:97, :4095] = out[:48, 1:4096]
    # patch means via 2D integral image of the channel-summed picture
    s = img.sum(0, dtype=np.float64)
    ii = np.zeros((H + 1, W + 1), np.float64)
    ii[1:, 1:] = np.cumsum(np.cumsum(s, 0), 1)
    h0 = np.arange(63) * 4
    win = (ii[np.ix_(h0 + 8, h0 + 8)] - ii[np.ix_(h0, h0 + 8)]
           - ii[np.ix_(h0 + 8, h0)] + ii[np.ix_(h0, h0)])
    mg = np.zeros((64, 64), np.float32)
    mg[:63, :63] = (win / D).astype(np.float32)
    out[48, :4096] = mg.reshape(-1).astype(np.float16)
    return out, mg


# ---------------------------------------------------------------- device program
def _enable_ldw_opt():
    """Flip walrus --enable-ldw-opt to true for this process's compiles so
    LDWEIGHTS can target the background weight buffer and hide behind the
    matmul stream."""
    import concourse.bass_utils as bu
    if getattr(bu, "_ldw_opt_patched", False):
        return
    orig = bu.run_command

    def run_command_ldw(argv, **kw):
        argv = ["--enable-ldw-opt=true" if a == "--enable-ldw-opt=false" else a
                for a in argv]
        return orig(argv, **kw)

    bu.run_command = run_command_ldw
    bu._ldw_opt_patched = True


def _build_program(thr):
    import concourse.tile as tile
    import concourse.mybir as mybir
    from concourse import bacc

    ssk = _softshrink_op()
    dt = mybir.dt
    f32, f16 = dt.float32, dt.float16

    nc = bacc.Bacc("TRN2", target_bir_lowering=False, debug=False,
                   num_devices=NCORES)
    PIECE = [0, 1152, 2176, 3200, Y4F]
    y4p_d = [nc.dram_tensor(f"y4p{i}", [97, PIECE[i + 1] - PIECE[i]], f16,
                            kind="ExternalInput").ap() for i in range(4)]
    m2_d = nc.dram_tensor("m2p", [128, 512], f16, kind="ExternalInput").ap()
    qlhs_d = nc.dram_tensor("qlhs2", [128, 512], f16, kind="ExternalInput").ap()
    anrec_d = nc.dram_tensor("anrec", [128, 384], f16, kind="ExternalInput").ap()
    out_d = {(fc, mb): nc.dram_tensor(f"out{fc}{mb}", [96, 1024], f32,
                                      kind="ExternalOutput").ap()
             for fc in range(4) for mb in range(2)}

    with tile.TileContext(nc) as tc:
        with tc.tile_pool(name="const", bufs=1) as cp:
            qlhs = cp.tile([128, 512], f16, tag="qlhs", name="qlhs_sb")
            m2 = cp.tile([128, 512], f16, tag="m2", name="m2_sb")
            anrec = cp.tile([128, 384], f16, tag="anrec", name="anrec_sb")
            y4 = cp.tile([97, Y4F], f16, tag="y4", name="y4_sb")

            qs = [cp.tile([128, NCOL], f32, tag=f"qs{mb}", name=f"qs{mb}_sb")
                  for mb in range(2)]
            c = [[cp.tile([128, 1024], f16, tag=f"c{kc}_{fc}",
                          name=f"c{kc}_{fc}_sb") for fc in range(4)]
                 for kc in range(2)]
            recT = [cp.tile([96, NCOL], f32, tag=f"recT{mb}",
                            name=f"recT{mb}_sb") for mb in range(2)]

            zeros = cp.tile([128, 1024], f32, tag="zeros", name="zeros_sb")
            nc.vector.memset(zeros[:], 0.0)

            # scratch for the ACT/GPSIMD soft-threshold flow (B-chunks)
            ab_sb = [[cp.tile([128, 1024], f16, tag=f"ab{i}_{j}",
                              name=f"ab{i}_{j}_sb") for j in range(2)]
                     for i in range(2)]
            nthr_b = cp.tile([128, 1], f32, tag="nthr", name="nthr_sb")
            nc.vector.memset(nthr_b[:], -thr)

            # B-chunks: soft-threshold via ACT relu-pair + GPSIMD subtract,
            # reading PSUM pre-seeded with qs (seed rides a DMA queue). The
            # PSUM tiles are persistent: their has_written bits are set once
            # by bootstrap matmuls, then only start=False matmuls touch them.
            B_CHUNKS = {(1, 1): 0, (3, 1): 1}  # (fc, mb) -> B-tile index

            psB_pool = tc.tile_pool(name="psB", bufs=1, space="PSUM")
            psB_cm = psB_pool.__enter__()
            psB = [psB_cm.tile([128, 1024], f32, tag=f"B{i}", name=f"B{i}_ps")
                   for i in range(2)]

            with tc.tile_pool(name="ps", bufs=2, space="PSUM") as pp:
                # ---- PE warm-up (no data deps, emitted before any DMA);
                # first 4 matmuls bootstrap the B-tiles' has_written bits ----
                wz = cp.tile([128, 512], f16, tag="wz", name="wz_sb")
                nc.vector.memset(wz[:].bitcast(f32), 0.0)
                for i in range(2):
                    for h in range(2):
                        nc.tensor.matmul(psB[i][:, h * 512:(h + 1) * 512],
                                         wz[:, 0:128], wz[:, 0:512],
                                         start=True, stop=True,
                                         skip_group_check=True)
                wps = pp.tile([128, 512], f32, tag="chunk", name="warm_ps")
                for w in range(6):
                    nc.tensor.matmul(wps[:], wz[:, 0:128], wz[:, 0:512],
                                     start=(w == 0), stop=(w == 5))

                # ---- input DMAs: the gpsimd DGE path fans packets out
                # across all 16 DMA engines; sync/scalar mostly serialize ----
                nc.scalar.dma_start(qlhs[:], qlhs_d[:])
                nc.scalar.dma_start(m2[:], m2_d[:])
                # one DMA engine serves one dma_start: split each piece into
                # row bands and fan them over the gpsimd + sync queues
                for i in range(4):
                    eng = [nc.gpsimd, nc.sync][i % 2]
                    eng.dma_start(y4[0:49, PIECE[i]:PIECE[i + 1]],
                                  y4p_d[i][0:49, :])
                    eng.dma_start(y4[49:97, PIECE[i]:PIECE[i + 1]],
                                  y4p_d[i][49:97, :])
                nc.scalar.dma_start(anrec[:], anrec_d[:])

                def shrink(fc, mb, ps, in1):
                    """c[mb][fc] = soft-threshold of (ps + in1)."""
                    bi = B_CHUNKS.get((fc, mb))
                    if bi is None:
                        nc.vector._custom_dve(
                            ssk, out=c[mb][fc][:], in0=ps[:], in1=in1,
                            s0=-thr, s1=thr)
                    else:
                        Relu = mybir.ActivationFunctionType.Relu
                        a_sb, b_sb = ab_sb[bi]
                        nc.scalar.activation(a_sb[:], ps[:], Relu,
                                             bias=nthr_b[:], scale=1.0)
                        nc.scalar.activation(b_sb[:], ps[:], Relu,
                                             bias=nthr_b[:], scale=-1.0)
                        nc.gpsimd.tensor_sub(c[mb][fc][:], a_sb[:], b_sb[:])

                # ---- phase Q: qs = (A p^T - arow x mean) / L, and the
                # matmul-free first ISTA iteration c1 = S_thr(qs) ----
                for fc in range(4):
                    for mb in range(2):
                        ps = pp.tile([128, 1024], f32, tag="chunk", name="q_ps")
                        # block A (carries the mean-correction row 48)
                        # first for both halves, then block B: adjacent
                        # matmuls share their stationary operand
                        for h in range(2):
                            col = fc * 1024 + h * 512
                            nc.tensor.matmul(
                                ps[:, h * 512:(h + 1) * 512],
                                qlhs[0:97, mb * 128:mb * 128 + 128],
                                y4[0:97, col:col + 512],
                                start=True, stop=False)
                        for h in range(2):
                            col = fc * 1024 + h * 512
                            nc.tensor.matmul(
                                ps[:, h * 512:(h + 1) * 512],
                                qlhs[0:97, 256 + mb * 128:256 + mb * 128 + 128],
                                y4[0:97, col + 64:col + 64 + 512],
                                start=False, stop=True)
                        nc.scalar.copy(qs[mb][:, fc * 1024:(fc + 1) * 1024],
                                       ps[:])
                        shrink(fc, mb, ps, zeros[:])

                # ---- ISTA iterations 2..NITER ----
                def ista_chunk(fc, mb):
                    """Emit matmuls for chunk (fc, mb)."""
                    bi = B_CHUNKS.get((fc, mb))
                    if bi is None:
                        ps = pp.tile([128, 1024], f32, tag="chunk",
                                     name="ista_ps")
                        seeded = False
                    else:
                        ps = psB[bi]
                        seeded = True
                    for kc in range(2):
                        for h in range(2):
                            nc.tensor.matmul(
                                ps[:, h * 512:(h + 1) * 512],
                                m2[:, (kc * 2 + mb) * 128:(kc * 2 + mb + 1) * 128],
                                c[kc][fc][:, h * 512:h * 512 + 512],
                                start=(kc == 0 and not seeded), stop=(kc == 1),
                                skip_group_check=seeded)
                    return ps

                def ista_iter(rec_tail=False):
                    # seed the B psum tiles with qs up front (ACT), so the
                    # in-order PE queue never waits on a seed: the B matmuls
                    # come last in the iteration.
                    for (fc, mb), bi in B_CHUNKS.items():
                        nc.scalar.copy(psB[bi][:],
                                       qs[mb][:, fc * 1024:(fc + 1) * 1024])
                    # pool-backed chunks interleaved with the psB-backed
                    # ones so a pool bank is never reclaimed before its DVE
                    # shrink (queued behind earlier shrinks) has drained it
                    for fc, mb in ((0, 0), (0, 1), (1, 1), (1, 0),
                                   (2, 0), (2, 1), (3, 1), (3, 0)):
                        ps = ista_chunk(fc, mb)
                        bi = B_CHUNKS.get((fc, mb))
                        in1 = zeros[:] if bi is not None else \
                            qs[mb][:, fc * 1024:(fc + 1) * 1024]
                        shrink(fc, mb, ps, in1)
                    if rec_tail:
                        for fc in (0, 2, 1, 3):
                            for mb in range(2):
                                rec_chunk(fc, mb)

                # ---- rec^T = A^T c (permuted d-axis); each chunk is copied
                # out of PSUM (ACT/DVE alternating) and DMA'd to DRAM as soon
                # as it is ready; mean + fold + counts happen on host ----
                def rec_chunk(fc, mb):
                    ps = pp.tile([96, 1024], f32, tag="chunk", name="rec_ps")
                    for kc in range(2):
                        for h in range(2):
                            nc.tensor.matmul(
                                ps[:, h * 512:(h + 1) * 512],
                                anrec[:, (kc * 2 + mb) * 96:(kc * 2 + mb + 1) * 96],
                                c[kc][fc][:, h * 512:h * 512 + 512],
                                start=(kc == 0), stop=(kc == 1))
                    dst = recT[mb][:, fc * 1024:(fc + 1) * 1024]
                    if (fc + mb) % 2 == 0:
                        nc.scalar.copy(dst, ps[:])
                    else:
                        nc.vector.tensor_copy(dst, ps[:])
                    nc.gpsimd.dma_start(out_d[(fc, mb)][0:48, :], dst[0:48, :])
                    nc.sync.dma_start(out_d[(fc, mb)][48:96, :], dst[48:96, :])

                for _ in range(NITER - 2):
                    ista_iter()
                ista_iter(rec_tail=True)  # last iteration: rec interleaved

            psB_pool.__exit__(None, None, None)

    nc.compile()
    return nc


# ---------------------------------------------------------------- entry point
def kernel(y, atoms):
    from concourse import bass_utils

    y = np.asarray(y, dtype=np.float32)
    atoms = np.asarray(atoms, dtype=np.float32)
    consts = _host_constants(atoms)
    thr = consts["thr"]

    key = round(thr, 12)
    if key not in _prog_cache:
        _prog_cache[key] = _build_program(thr)
    nc = _prog_cache[key]

    shared = {"m2p": consts["m2p"], "qlhs2": consts["qlhs2"],
              "anrec": consts["anrec"]}
    packed = [_make_y4(y[b]) for b in range(B)]
    PIECE = [0, 1152, 2176, 3200, Y4F]
    in_maps = [dict(shared, **{f"y4p{i}": np.ascontiguousarray(
        packed[b][0][:, PIECE[i]:PIECE[i + 1]]) for i in range(4)})
        for b in range(B)]
    res = bass_utils.run_bass_kernel_spmd(nc, in_maps,
                                          core_ids=list(range(NCORES)))
    out = np.empty((B, C, H, W), np.float32)
    inv = (1.0 / consts["counts"]).astype(np.float32)
    for b in range(B):
        r = res.results[b]
        dev = np.empty((192, NCOL), np.float32)
        for fc in range(4):
            for mb in range(2):
                dev[mb * 96:(mb + 1) * 96, fc * 1024:(fc + 1) * 1024] = \
                    r[f"out{fc}{mb}"]
        mg = packed[b][1]
        # fold: 4 shifted quadrant adds in the permuted [48, 64, 64] space,
        # with the per-patch mean added per contributing quadrant
        acc = np.zeros((48, 64, 64), np.float32)
        for a in range(2):
            for bq in range(2):
                quad = dev[a * 96 + 48 * bq: a * 96 + 48 * bq + 48]
                acc[:, a:a + 63, bq:bq + 63] += (
                    quad.reshape(48, 64, 64)[:, :63, :63] + mg[None, :63, :63])
        img = acc.reshape(4, 4, 3, 64, 64).transpose(2, 3, 0, 4, 1) \
            .reshape(3, 256, 256)
        out[b] = img * inv
    return out
